# revision 23
# baseline (speedup 1.0000x reference)
"""CSWin attention kernel for 8 trn2 NeuronCores — v3.

Strategy vs v2 (292us baseline):
  - host pre-casts qkv to f16 AND pre-transposes per window:
      qT/kT   [c=128, win, s=512]   (channel-major, contiguous big DMAs)
      vtok    [tok128, win, jc, c]  (token-major for AV stationary)
      vpad    [c, win, 662]         (host-padded LePE image, zero ring)
    -> no on-device casts, no PE transposes, 4-5 big contiguous input
       DMAs instead of 192 strided per-chunk DMAs (was ~118us of
       engine trigger time).
  - QK^T per chunk-half into PSUM f32 [128,1024] (2 heads row-tiled).
  - exp split across engines: ScalarE activation(Exp) for 5 of 8
    half-tiles per window, DVE Schraudolph bit-trick (i16 = s*A+B
    viewed as f16) for the other 3 -> breaks the 131us ScalarE floor.
  - AV + den(ones-matmul) accumulated over chunks in PSUM col-tiled.
  - LePE 3x3 depthwise conv: 9 diagonal-matmul taps with STRIDED
    moving view (exactly 512 interior cols, one phase, one PSUM bank);
    bias added on host.
  - outputs copied PSUM->SBUF f16 (DVE/ScalarE; GPSIMD can't touch
    PSUM) and DMA'd f16; host does att = av/den + lepe + bias and the
    window->image transpose.

PSUM budget: scores 2x[128,1024](4 banks) + av(1) + den(1) + lepe(2) = 8.
"""

import numpy as np

import concourse.bass as bass
import concourse.tile as tile
from concourse import mybir
from concourse.vector_clock import ScopedClock

RES = 64
SPLIT = 8
C = 128
HEADS = 4
HD = 32
S = RES * SPLIT          # 512 tokens per window
SCALE = HD ** -0.5
B = 16
N_CORES = 8
IMGS_PER_CORE = B // N_CORES   # 2
NWIN_IMG = RES // SPLIT        # 8 windows per image
NW = IMGS_PER_CORE * NWIN_IMG  # 16 windows per core
NCHUNK = S // 128              # 4 token-chunks per window

F32 = mybir.dt.float32
F16 = mybir.dt.float16
I16 = mybir.dt.int16

LOG2E = 1.4426950408889634
# Schraudolph f16: i16 = s * A + B, bit pattern of ~exp(s*SCALE)
SCH_A = float(SCALE * LOG2E * 1024.0)
SCH_B = float(15 * 1024 - 45 + 0.5)

# LePE padded image geometry (host-built): cell(y,x) = 12 + y*10 + x
XP = SPLIT + 2                 # 10
PADN = XP * (RES + 2) + 2      # 662
P0 = 12

# which half-tiles (idx = 2*jc + half, 0..7) use the DVE Schraudolph exp
DVE_HALVES = frozenset({1, 6})
TAPS = [(dy, dx) for dy in (-1, 0, 1) for dx in (-1, 0, 1)]
# LePE tap assignment: DVE and GPSIMD accumulate in SBUF f16, PE in PSUM
DVE_TAPS = (0, 1)
GP_TAPS = (2, 3)
PE_TAPS = tuple(t for t in range(9) if t not in DVE_TAPS + GP_TAPS)


# ---------------------------------------------------------------- compat ----

def _patched_drain_and_barrier(self, tick_clock, wait_clock):
    nc = self.nc
    nop_inst = nc.sync.nop(nofuse=True)
    wait_clock.add_sem_waits(nop_inst.ins, ScopedClock({None: tick_clock.global_clock}))
    si = nop_inst.ins.sync_info
    waits = list(si.on_wait) if si is not None else []
    if len(waits) > 1:
        si.on_wait = [waits[0]]
        for w in waits[1:]:
            n2 = nc.sync.nop(nofuse=True)
            n2.ins.sync_info = mybir.SyncInfo(on_wait=[w], on_update=[])
    nc.sync.drain()
    nc.all_engine_barrier()
    assert self.sems is not None
    popped = nc._tile_sem_poison_stack.pop()
    assert popped is self._sem_poison
    nc.clear_and_free_semaphores(list(self.sems.allocated().values()))
    nc.all_engine_barrier()


def _install_tile_patch():
    tile.TileContext._drain_and_barrier = _patched_drain_and_barrier


def _split_multiwaits(nc):
    """Hoist extra sync waits onto same-engine NOPs inserted just before the
    owning instruction (this walrus build allows 1 wait per instruction)."""
    for f in nc.m.functions:
        for bb in f.blocks:
            insts = bb.instructions
            if not any(
                i.sync_info is not None and len(i.sync_info.on_wait) > 1
                for i in insts
            ):
                continue
            new_insts = []
            for inst in insts:
                si = inst.sync_info
                if si is not None and len(si.on_wait) > 1:
                    waits = list(si.on_wait)
                    for w in waits[:-1]:
                        nop = mybir.InstNoOp(
                            name=nc.get_next_instruction_name(), ins=[], outs=[]
                        )
                        nop.engine = inst.engine
                        nop.sync_info = mybir.SyncInfo(on_wait=[w], on_update=[])
                        new_insts.append(nop)
                    si.on_wait = [waits[-1]]
                new_insts.append(inst)
            bb.instructions = new_insts


# ---------------------------------------------------------------- device ----

def _build_nc():
    _install_tile_patch()
    nc = bass.Bass(trn_type="TRN2", num_devices=N_CORES)

    qT_d = nc.dram_tensor("qT", [C, NW * S], F16, kind="ExternalInput")
    kT_d = nc.dram_tensor("kT", [C, NW * S], F16, kind="ExternalInput")
    vtok_d = nc.dram_tensor("vtok", [128, NW * NCHUNK * C], F16, kind="ExternalInput")
    vpad_d = nc.dram_tensor("vpad", [C, NW * PADN], F16, kind="ExternalInput")
    diag_d = nc.dram_tensor("diag", [C, 9 * C], F16, kind="ExternalInput")

    av_d = nc.dram_tensor("avT", [NW, C, S], F16, kind="ExternalOutput")
    den_d = nc.dram_tensor("denT", [NW, HEADS, S], F16, kind="ExternalOutput")
    lp_d = nc.dram_tensor("lepeT", [NW, C, S], F16, kind="ExternalOutput")

    wtap_d = nc.dram_tensor("wtap", [C, 9], F32, kind="ExternalInput")

    GRP = 2  # windows per input-DMA group

    with tile.TileContext(nc) as tc:
        with (
            tc.tile_pool(name="const", bufs=1) as const,
            tc.tile_pool(name="inp", bufs=1) as inp,
            tc.tile_pool(name="expt", bufs=4) as expt,
            tc.tile_pool(name="sbout", bufs=2) as sbout,
            tc.tile_pool(name="lacc", bufs=2) as lacc,
            tc.tile_pool(name="scoresp", bufs=2, space="PSUM") as scoresp,
            tc.tile_pool(name="avp", bufs=2, space="PSUM") as avp,
            tc.tile_pool(name="denp", bufs=1, space="PSUM") as denp,
            tc.tile_pool(name="lepep", bufs=1, space="PSUM") as lepep,
        ):
            diag_sb = const.tile([C, 9, C], F16)
            nc.sync.dma_start(
                out=diag_sb, in_=diag_d.ap().rearrange("c (t m) -> c t m", t=9)
            )
            ones_sb = const.tile([128, 1], F16)
            nc.vector.memset(ones_sb, 1.0)
            wtap_sb = const.tile([C, 9], F32)
            nc.sync.dma_start(out=wtap_sb, in_=wtap_d.ap())

            qT_sb = inp.tile([C, NW * S], F16)
            kT_sb = inp.tile([C, NW * S], F16)
            vtok_sb = inp.tile([128, NW * NCHUNK * C], F16)
            vpad_sb = inp.tile([C, NW * PADN], F16)
            for g in range(NW // GRP):
                for t_sb, t_d, width in (
                    (qT_sb, qT_d, S),
                    (kT_sb, kT_d, S),
                    (vtok_sb, vtok_d, NCHUNK * C),
                    (vpad_sb, vpad_d, PADN),
                ):
                    lo, hi = g * GRP * width, (g + 1) * GRP * width
                    nc.sync.dma_start(out=t_sb[:, lo:hi], in_=t_d.ap()[:, lo:hi])

            def vpad_view(w, t):
                dy, dx = TAPS[t]
                d = XP * dy + dx
                return bass.AP(
                    tensor=vpad_sb.tensor,
                    offset=vpad_sb.offset + w * PADN + P0 + d,
                    ap=[vpad_sb.ap[0], [XP, RES], [1, SPLIT]],
                )

            def pe_tap(w, lp, i):
                t = PE_TAPS[i]
                nc.tensor.matmul(
                    lp,
                    diag_sb[:, t, :],
                    vpad_view(w, t),
                    start=(i == 0),
                    stop=(i == len(PE_TAPS) - 1),
                    skip_group_check=True,
                )

            def dve_taps(w):
                acc = lacc.tile([128, S], F16, tag="lacc")
                for i, t in enumerate(DVE_TAPS):
                    if i == 0:
                        nc.vector.tensor_scalar(
                            out=acc,
                            in0=vpad_view(w, t),
                            scalar1=wtap_sb[:, t : t + 1],
                            scalar2=None,
                            op0=mybir.AluOpType.mult,
                        )
                    else:
                        nc.vector.scalar_tensor_tensor(
                            out=acc,
                            in0=vpad_view(w, t),
                            scalar=wtap_sb[:, t : t + 1],
                            in1=acc,
                            op0=mybir.AluOpType.mult,
                            op1=mybir.AluOpType.add,
                        )
                return acc

            def gp_taps(w):
                t0, t1 = GP_TAPS
                tmp0 = lacc.tile([128, S], F16, tag="gtmp0")
                nc.gpsimd.tensor_scalar(
                    out=tmp0,
                    in0=vpad_view(w, t0),
                    scalar1=wtap_sb[:, t0 : t0 + 1],
                    scalar2=None,
                    op0=mybir.AluOpType.mult,
                )
                tmp1 = lacc.tile([128, S], F16, tag="gtmp1")
                nc.gpsimd.tensor_scalar(
                    out=tmp1,
                    in0=vpad_view(w, t1),
                    scalar1=wtap_sb[:, t1 : t1 + 1],
                    scalar2=None,
                    op0=mybir.AluOpType.mult,
                )
                gacc = lacc.tile([128, S], F16, tag="gacc")
                nc.gpsimd.tensor_tensor(
                    out=gacc, in0=tmp0, in1=tmp1, op=mybir.AluOpType.add
                )
                return gacc

            def lepe_out(w, lp, acc, gacc):
                # lepe = PE part (PSUM f32) + DVE part + GPSIMD part (SBUF f16)
                lp_sb = sbout.tile([128, S], F16, tag="lp_sb")
                nc.vector.scalar_tensor_tensor(
                    out=lp_sb,
                    in0=lp,
                    scalar=1.0,
                    in1=acc,
                    op0=mybir.AluOpType.mult,
                    op1=mybir.AluOpType.add,
                )
                nc.vector.tensor_tensor(
                    out=lp_sb, in0=lp_sb, in1=gacc, op=mybir.AluOpType.add
                )
                nc.gpsimd.dma_start(out=lp_d.ap()[w], in_=lp_sb)

            prev_lp = None  # (w, lp, acc, gacc) of the previous window

            for w in range(NW):
                av_ps = avp.tile([128, S], F32, tag="av")
                den_ps = denp.tile([128, S], F32, tag="den")
                ets = {}

                def qk_exp(jc, half, w=w, ets=None):
                    st = scoresp.tile([128, 2 * S], F32, tag="st")
                    for hh in range(2):
                        h = 2 * half + hh
                        hp = 32 * h
                        nc.tensor.matmul(
                            st[:, S * hh : S * (hh + 1)],
                            kT_sb[hp : hp + 32, w * S + 128 * jc : w * S + 128 * (jc + 1)],
                            qT_sb[hp : hp + 32, w * S : (w + 1) * S],
                            start=True,
                            stop=True,
                            tile_position=(hp, 0),
                        )
                    et = expt.tile([128, 2 * S], F16, tag="et")
                    idx = 2 * jc + half
                    if idx in DVE_HALVES:
                        nc.vector.tensor_scalar(
                            out=et.bitcast(I16),
                            in0=st,
                            scalar1=SCH_A,
                            scalar2=SCH_B,
                            op0=mybir.AluOpType.mult,
                            op1=mybir.AluOpType.add,
                        )
                    else:
                        nc.scalar.activation(
                            out=et,
                            in_=st,
                            func=mybir.ActivationFunctionType.Exp,
                            scale=float(SCALE),
                        )
                    ets[idx] = et

                def av_den(jc, w=w, ets=None):
                    for h in range(HEADS):
                        hp = 32 * h
                        et_h = ets[2 * jc + h // 2][:, S * (h % 2) : S * (h % 2 + 1)]
                        nc.tensor.matmul(
                            av_ps[hp : hp + 32, :],
                            vtok_sb[:, w * S + jc * C + hp : w * S + jc * C + hp + 32],
                            et_h,
                            start=(jc == 0),
                            stop=(jc == NCHUNK - 1),
                            tile_position=(0, hp),
                            skip_group_check=True,
                        )
                    for h in range(HEADS):
                        hp = 32 * h
                        et_h = ets[2 * jc + h // 2][:, S * (h % 2) : S * (h % 2 + 1)]
                        nc.tensor.matmul(
                            den_ps[hp : hp + 1, :],
                            ones_sb,
                            et_h,
                            start=(jc == 0),
                            stop=(jc == NCHUNK - 1),
                            tile_position=(0, hp),
                            skip_group_check=True,
                        )

                # drain previous window's lepe (DVE combine + DMA, no PE work)
                if prev_lp is not None:
                    lepe_out(*prev_lp)

                qk_exp(0, 0, ets=ets)
                qk_exp(0, 1, ets=ets)
                qk_exp(1, 0, ets=ets)
                qk_exp(1, 1, ets=ets)
                av_den(0, ets=ets)
                # PE lepe taps interleaved between independent matmuls so the
                # PSUM-accumulate RAW bubble is absorbed by other streams
                lp = lepep.tile([128, S], F32, tag="lp")
                pe_tap(w, lp, 0)
                qk_exp(2, 0, ets=ets)
                pe_tap(w, lp, 1)
                qk_exp(2, 1, ets=ets)
                av_den(1, ets=ets)
                pe_tap(w, lp, 2)
                qk_exp(3, 0, ets=ets)
                pe_tap(w, lp, 3)
                qk_exp(3, 1, ets=ets)
                av_den(2, ets=ets)
                pe_tap(w, lp, 4)
                gacc = gp_taps(w)
                av_den(3, ets=ets)

                # ---- drain PSUM -> SBUF f16 -> DRAM (den first: bufs=1) ----
                den_sb = sbout.tile([128, S], F16, tag="den_sb")
                nc.vector.tensor_copy(out=den_sb, in_=den_ps)
                den_view = bass.AP(
                    tensor=den_sb.tensor,
                    offset=den_sb.offset,
                    ap=[[den_sb.ap[0][0] * 32, 4], [1, S]],
                )
                nc.gpsimd.dma_start(out=den_d.ap()[w], in_=den_view)

                av_sb = sbout.tile([128, S], F16, tag="av_sb")
                nc.vector.tensor_copy(out=av_sb, in_=av_ps)
                nc.gpsimd.dma_start(out=av_d.ap()[w], in_=av_sb)

                acc = dve_taps(w)
                prev_lp = (w, lp, acc, gacc)

            lepe_out(*prev_lp)

    _split_multiwaits(nc)
    return nc


# ------------------------------------------------------------------ host ----

_NC_CACHE = {}


def _get_nc():
    if "nc" not in _NC_CACHE:
        _NC_CACHE["nc"] = _build_nc()
    return _NC_CACHE["nc"]


def _host_prep(qkv, conv_w):
    """Build per-core input arrays (all f16)."""
    f16 = np.float16
    # [3, B, 4096, 128] -> window grids [3, B, y, sx, x, c]
    qkv_w = qkv.reshape(3, B, RES, NWIN_IMG, SPLIT, C)

    cores = []
    for core in range(N_CORES):
        bs = slice(core * IMGS_PER_CORE, (core + 1) * IMGS_PER_CORE)
        q = qkv_w[0, bs]   # [2, y, sx, x, c]
        k = qkv_w[1, bs]
        v = qkv_w[2, bs]

        # [c, img, sx, y, x] -> [128, NW*512]
        qT = np.ascontiguousarray(q.transpose(4, 0, 2, 1, 3)).reshape(C, NW * S)
        kT = np.ascontiguousarray(k.transpose(4, 0, 2, 1, 3)).reshape(C, NW * S)

        # v token-major: [yy, x, img, sx, jc, c] -> [128, NW*4*128]
        vt = v.reshape(IMGS_PER_CORE, NCHUNK, 16, NWIN_IMG, SPLIT, C)
        vtok = np.ascontiguousarray(vt.transpose(2, 4, 0, 3, 1, 5)).reshape(
            128, NW * NCHUNK * C
        )

        # vpad: [c, win, 662] with interior at 12 + y*10 + x
        vimg = np.ascontiguousarray(v.transpose(4, 0, 2, 1, 3))  # [c, img, sx, y, x]
        vpad = np.zeros((C, NW, PADN), dtype=f16)
        vpad_v = vpad[:, :, 1:661].reshape(C, NW, RES + 2, XP)
        vpad_v[:, :, 1:-1, 1:-1] = vimg.reshape(C, NW, RES, SPLIT)

        cores.append(
            {
                "qT": qT.astype(f16),
                "kT": kT.astype(f16),
                "vtok": vtok.astype(f16),
                "vpad": vpad.reshape(C, NW * PADN),
            }
        )

    w9 = conv_w.reshape(C, 9).astype(np.float32)
    diag = np.zeros((C, 9, C), dtype=np.float32)
    idx = np.arange(C)
    for t in range(9):
        diag[idx, t, idx] = w9[:, t]
    diag = diag.reshape(C, 9 * C).astype(f16)
    for m in cores:
        m["diag"] = diag
        m["wtap"] = w9
    return cores


def kernel(qkv, conv_w, conv_b):
    from concourse.bass_utils import run_bass_kernel_spmd

    qkv = np.asarray(qkv, dtype=np.float32)
    conv_w = np.asarray(conv_w, np.float32)
    conv_b = np.asarray(conv_b, np.float32)

    nc = _get_nc()
    in_maps = _host_prep(qkv, conv_w)

    res = run_bass_kernel_spmd(nc, in_maps, core_ids=list(range(N_CORES)))
    global LAST_RESULT
    LAST_RESULT = res

    outs = []
    for r in res.results:
        av = r["avT"].astype(np.float32)       # [16, 128, 512]
        den = r["denT"].astype(np.float32)     # [16, 4, 512]
        lp = r["lepeT"].astype(np.float32)     # [16, 128, 512]
        att = av.reshape(NW, HEADS, HD, S) / den.reshape(NW, HEADS, 1, S)
        o = att.reshape(NW, C, S) + lp + conv_b.astype(np.float32)[None, :, None]
        # [win, c, s] -> [img, y, x, c]
        o = o.reshape(IMGS_PER_CORE, NWIN_IMG, C, RES, SPLIT)
        o = o.transpose(0, 3, 1, 4, 2).reshape(IMGS_PER_CORE, RES, RES, C)
        outs.append(o)
    return np.concatenate(outs, axis=0)


LAST_RESULT = None


# revision 27
# speedup vs baseline: 1.4387x; 1.4387x over previous
"""CSWin attention kernel for 8 trn2 NeuronCores — v3.

Strategy vs v2 (292us baseline):
  - host pre-casts qkv to f16 AND pre-transposes per window:
      qT/kT   [c=128, win, s=512]   (channel-major, contiguous big DMAs)
      vtok    [tok128, win, jc, c]  (token-major for AV stationary)
      vpad    [c, win, 662]         (host-padded LePE image, zero ring)
    -> no on-device casts, no PE transposes, 4-5 big contiguous input
       DMAs instead of 192 strided per-chunk DMAs (was ~118us of
       engine trigger time).
  - QK^T per chunk-half into PSUM f32 [128,1024] (2 heads row-tiled).
  - exp split across engines: ScalarE activation(Exp) for 5 of 8
    half-tiles per window, DVE Schraudolph bit-trick (i16 = s*A+B
    viewed as f16) for the other 3 -> breaks the 131us ScalarE floor.
  - AV + den(ones-matmul) accumulated over chunks in PSUM col-tiled.
  - LePE 3x3 depthwise conv: 9 diagonal-matmul taps with STRIDED
    moving view (exactly 512 interior cols, one phase, one PSUM bank);
    bias added on host.
  - outputs copied PSUM->SBUF f16 (DVE/ScalarE; GPSIMD can't touch
    PSUM) and DMA'd f16; host does att = av/den + lepe + bias and the
    window->image transpose.

PSUM budget: scores 2x[128,1024](4 banks) + av(1) + den(1) + lepe(2) = 8.
"""

import numpy as np

import concourse.bass as bass
import concourse.tile as tile
from concourse import mybir
from concourse.vector_clock import ScopedClock

RES = 64
SPLIT = 8
C = 128
HEADS = 4
HD = 32
S = RES * SPLIT          # 512 tokens per window
SCALE = HD ** -0.5
B = 16
N_CORES = 8
IMGS_PER_CORE = B // N_CORES   # 2
NWIN_IMG = RES // SPLIT        # 8 windows per image
NW = IMGS_PER_CORE * NWIN_IMG  # 16 windows per core
NCHUNK = S // 128              # 4 token-chunks per window

F32 = mybir.dt.float32
F16 = mybir.dt.float16
I16 = mybir.dt.int16

LOG2E = 1.4426950408889634
# Schraudolph f16: i16 = s * A + B, bit pattern of ~exp(s*SCALE)
SCH_A = float(SCALE * LOG2E * 1024.0)
SCH_B = float(15 * 1024 - 45 + 0.5)

# LePE padded image geometry (host-built): cell(y,x) = 12 + y*10 + x
XP = SPLIT + 2                 # 10
PADN = XP * (RES + 2) + 2      # 662
P0 = 12

# which half-tiles (idx = 2*jc + half, 0..7) use the DVE Schraudolph exp
DVE_HALVES = frozenset({1, 6})
TAPS = [(dy, dx) for dy in (-1, 0, 1) for dx in (-1, 0, 1)]
# LePE tap assignment: DVE accumulates in SBUF f16, PE in PSUM
DVE_TAPS = (0, 1, 2)
PE_TAPS = tuple(t for t in range(9) if t not in DVE_TAPS)


# ---------------------------------------------------------------- compat ----

def _patched_drain_and_barrier(self, tick_clock, wait_clock):
    nc = self.nc
    nop_inst = nc.sync.nop(nofuse=True)
    wait_clock.add_sem_waits(nop_inst.ins, ScopedClock({None: tick_clock.global_clock}))
    si = nop_inst.ins.sync_info
    waits = list(si.on_wait) if si is not None else []
    if len(waits) > 1:
        si.on_wait = [waits[0]]
        for w in waits[1:]:
            n2 = nc.sync.nop(nofuse=True)
            n2.ins.sync_info = mybir.SyncInfo(on_wait=[w], on_update=[])
    nc.sync.drain()
    nc.all_engine_barrier()
    assert self.sems is not None
    popped = nc._tile_sem_poison_stack.pop()
    assert popped is self._sem_poison
    nc.clear_and_free_semaphores(list(self.sems.allocated().values()))
    nc.all_engine_barrier()


def _install_tile_patch():
    tile.TileContext._drain_and_barrier = _patched_drain_and_barrier


def _split_multiwaits(nc):
    """Hoist extra sync waits onto same-engine NOPs inserted just before the
    owning instruction (this walrus build allows 1 wait per instruction)."""
    for f in nc.m.functions:
        for bb in f.blocks:
            insts = bb.instructions
            if not any(
                i.sync_info is not None and len(i.sync_info.on_wait) > 1
                for i in insts
            ):
                continue
            new_insts = []
            for inst in insts:
                si = inst.sync_info
                if si is not None and len(si.on_wait) > 1:
                    waits = list(si.on_wait)
                    for w in waits[:-1]:
                        nop = mybir.InstNoOp(
                            name=nc.get_next_instruction_name(), ins=[], outs=[]
                        )
                        nop.engine = inst.engine
                        nop.sync_info = mybir.SyncInfo(on_wait=[w], on_update=[])
                        new_insts.append(nop)
                    si.on_wait = [waits[-1]]
                new_insts.append(inst)
            bb.instructions = new_insts


# ---------------------------------------------------------------- device ----

def _build_nc():
    _install_tile_patch()
    nc = bass.Bass(trn_type="TRN2", num_devices=N_CORES)

    qT_d = nc.dram_tensor("qT", [C, NW * S], F16, kind="ExternalInput")
    kT_d = nc.dram_tensor("kT", [C, NW * S], F16, kind="ExternalInput")
    vtok_d = nc.dram_tensor("vtok", [128, NW * NCHUNK * C], F16, kind="ExternalInput")
    vpad_d = nc.dram_tensor("vpad", [C, NW * PADN], F16, kind="ExternalInput")
    diag_d = nc.dram_tensor("diag", [C, 9 * C], F16, kind="ExternalInput")

    av_d = nc.dram_tensor("avT", [NW, C, S], F16, kind="ExternalOutput")
    den_d = nc.dram_tensor("denT", [NW, HEADS, S], F16, kind="ExternalOutput")
    lp_d = nc.dram_tensor("lepeT", [NW, C, S], F16, kind="ExternalOutput")

    wtap_d = nc.dram_tensor("wtap", [C, 9], F32, kind="ExternalInput")

    GRP = 2  # windows per input-DMA group

    with tile.TileContext(nc) as tc:
        with (
            tc.tile_pool(name="const", bufs=1) as const,
            tc.tile_pool(name="inp", bufs=1) as inp,
            tc.tile_pool(name="expt", bufs=4) as expt,
            tc.tile_pool(name="sbout", bufs=2) as sbout,
            tc.tile_pool(name="lacc", bufs=2) as lacc,
            tc.tile_pool(name="scoresp", bufs=2, space="PSUM") as scoresp,
            tc.tile_pool(name="avp", bufs=2, space="PSUM") as avp,
            tc.tile_pool(name="denp", bufs=1, space="PSUM") as denp,
            tc.tile_pool(name="lepep", bufs=1, space="PSUM") as lepep,
        ):
            diag_sb = const.tile([C, 9, C], F16)
            nc.sync.dma_start(
                out=diag_sb, in_=diag_d.ap().rearrange("c (t m) -> c t m", t=9)
            )
            ones_sb = const.tile([128, 1], F16)
            nc.vector.memset(ones_sb, 1.0)
            wtap_sb = const.tile([C, 9], F32)
            nc.sync.dma_start(out=wtap_sb, in_=wtap_d.ap())

            qT_sb = inp.tile([C, NW * S], F16)
            kT_sb = inp.tile([C, NW * S], F16)
            vtok_sb = inp.tile([128, NW * NCHUNK * C], F16)
            vpad_sb = inp.tile([C, NW * PADN], F16)
            for g in range(NW // GRP):
                for t_sb, t_d, width in (
                    (qT_sb, qT_d, S),
                    (kT_sb, kT_d, S),
                    (vtok_sb, vtok_d, NCHUNK * C),
                    (vpad_sb, vpad_d, PADN),
                ):
                    lo, hi = g * GRP * width, (g + 1) * GRP * width
                    nc.sync.dma_start(out=t_sb[:, lo:hi], in_=t_d.ap()[:, lo:hi])

            def vpad_view(w, t):
                dy, dx = TAPS[t]
                d = XP * dy + dx
                return bass.AP(
                    tensor=vpad_sb.tensor,
                    offset=vpad_sb.offset + w * PADN + P0 + d,
                    ap=[vpad_sb.ap[0], [XP, RES], [1, SPLIT]],
                )

            def pe_tap(w, lp, i):
                t = PE_TAPS[i]
                nc.tensor.matmul(
                    lp,
                    diag_sb[:, t, :],
                    vpad_view(w, t),
                    start=(i == 0),
                    stop=(i == len(PE_TAPS) - 1),
                    skip_group_check=True,
                )

            def dve_taps(w):
                acc = lacc.tile([128, S], F16, tag="lacc")
                for i, t in enumerate(DVE_TAPS):
                    if i == 0:
                        nc.vector.tensor_scalar(
                            out=acc,
                            in0=vpad_view(w, t),
                            scalar1=wtap_sb[:, t : t + 1],
                            scalar2=None,
                            op0=mybir.AluOpType.mult,
                        )
                    else:
                        nc.vector.scalar_tensor_tensor(
                            out=acc,
                            in0=vpad_view(w, t),
                            scalar=wtap_sb[:, t : t + 1],
                            in1=acc,
                            op0=mybir.AluOpType.mult,
                            op1=mybir.AluOpType.add,
                        )
                return acc

            def lepe_out(w, lp, acc):
                # lepe = PE part (PSUM f32) + DVE part (SBUF f16)
                lp_sb = sbout.tile([128, S], F16, tag="lp_sb")
                nc.vector.scalar_tensor_tensor(
                    out=lp_sb,
                    in0=lp,
                    scalar=1.0,
                    in1=acc,
                    op0=mybir.AluOpType.mult,
                    op1=mybir.AluOpType.add,
                )
                nc.gpsimd.dma_start(out=lp_d.ap()[w], in_=lp_sb)

            prev_lp = None  # (w, lp, acc) of the previous window

            for w in range(NW):
                av_ps = avp.tile([128, S], F32, tag="av")
                den_ps = denp.tile([128, S], F32, tag="den")
                ets = {}

                def qk_exp(jc, half, w=w, ets=None):
                    st = scoresp.tile([128, 2 * S], F32, tag="st")
                    for hh in range(2):
                        h = 2 * half + hh
                        hp = 32 * h
                        nc.tensor.matmul(
                            st[:, S * hh : S * (hh + 1)],
                            kT_sb[hp : hp + 32, w * S + 128 * jc : w * S + 128 * (jc + 1)],
                            qT_sb[hp : hp + 32, w * S : (w + 1) * S],
                            start=True,
                            stop=True,
                            tile_position=(hp, 0),
                        )
                    et = expt.tile([128, 2 * S], F16, tag="et")
                    idx = 2 * jc + half
                    if idx in DVE_HALVES:
                        nc.vector.tensor_scalar(
                            out=et.bitcast(I16),
                            in0=st,
                            scalar1=SCH_A,
                            scalar2=SCH_B,
                            op0=mybir.AluOpType.mult,
                            op1=mybir.AluOpType.add,
                        )
                    else:
                        nc.scalar.activation(
                            out=et,
                            in_=st,
                            func=mybir.ActivationFunctionType.Exp,
                            scale=float(SCALE),
                        )
                    ets[idx] = et

                def av_den(jc, w=w, ets=None):
                    for h in range(HEADS):
                        hp = 32 * h
                        et_h = ets[2 * jc + h // 2][:, S * (h % 2) : S * (h % 2 + 1)]
                        nc.tensor.matmul(
                            av_ps[hp : hp + 32, :],
                            vtok_sb[:, w * S + jc * C + hp : w * S + jc * C + hp + 32],
                            et_h,
                            start=(jc == 0),
                            stop=(jc == NCHUNK - 1),
                            tile_position=(0, hp),
                            skip_group_check=True,
                        )
                    for h in range(HEADS):
                        hp = 32 * h
                        et_h = ets[2 * jc + h // 2][:, S * (h % 2) : S * (h % 2 + 1)]
                        nc.tensor.matmul(
                            den_ps[hp : hp + 1, :],
                            ones_sb,
                            et_h,
                            start=(jc == 0),
                            stop=(jc == NCHUNK - 1),
                            tile_position=(0, hp),
                            skip_group_check=True,
                        )

                # drain previous window's lepe (DVE combine + DMA, no PE work)
                if prev_lp is not None:
                    lepe_out(*prev_lp)

                qk_exp(0, 0, ets=ets)
                qk_exp(0, 1, ets=ets)
                qk_exp(1, 0, ets=ets)
                qk_exp(1, 1, ets=ets)
                av_den(0, ets=ets)
                # PE lepe taps interleaved between independent matmuls so the
                # PSUM-accumulate RAW bubble is absorbed by other streams
                lp = lepep.tile([128, S], F32, tag="lp")
                pe_tap(w, lp, 0)
                qk_exp(2, 0, ets=ets)
                pe_tap(w, lp, 1)
                qk_exp(2, 1, ets=ets)
                av_den(1, ets=ets)
                pe_tap(w, lp, 2)
                qk_exp(3, 0, ets=ets)
                pe_tap(w, lp, 3)
                qk_exp(3, 1, ets=ets)
                av_den(2, ets=ets)
                pe_tap(w, lp, 4)
                pe_tap(w, lp, 5)
                av_den(3, ets=ets)

                # ---- drain PSUM -> SBUF f16 -> DRAM (den first: bufs=1) ----
                den_sb = sbout.tile([128, S], F16, tag="den_sb")
                nc.vector.tensor_copy(out=den_sb, in_=den_ps)
                den_view = bass.AP(
                    tensor=den_sb.tensor,
                    offset=den_sb.offset,
                    ap=[[den_sb.ap[0][0] * 32, 4], [1, S]],
                )
                nc.gpsimd.dma_start(out=den_d.ap()[w], in_=den_view)

                av_sb = sbout.tile([128, S], F16, tag="av_sb")
                nc.vector.tensor_copy(out=av_sb, in_=av_ps)
                nc.gpsimd.dma_start(out=av_d.ap()[w], in_=av_sb)

                acc = dve_taps(w)
                prev_lp = (w, lp, acc)

            lepe_out(*prev_lp)

    _split_multiwaits(nc)
    return nc


# ------------------------------------------------------------------ host ----

_NC_CACHE = {}


def _get_nc():
    if "nc" not in _NC_CACHE:
        _NC_CACHE["nc"] = _build_nc()
    return _NC_CACHE["nc"]


def _host_prep(qkv, conv_w):
    """Build per-core input arrays (all f16)."""
    f16 = np.float16
    # [3, B, 4096, 128] -> window grids [3, B, y, sx, x, c]
    qkv_w = qkv.reshape(3, B, RES, NWIN_IMG, SPLIT, C)

    cores = []
    for core in range(N_CORES):
        bs = slice(core * IMGS_PER_CORE, (core + 1) * IMGS_PER_CORE)
        q = qkv_w[0, bs]   # [2, y, sx, x, c]
        k = qkv_w[1, bs]
        v = qkv_w[2, bs]

        # [c, img, sx, y, x] -> [128, NW*512]
        qT = np.ascontiguousarray(q.transpose(4, 0, 2, 1, 3)).reshape(C, NW * S)
        kT = np.ascontiguousarray(k.transpose(4, 0, 2, 1, 3)).reshape(C, NW * S)

        # v token-major: [yy, x, img, sx, jc, c] -> [128, NW*4*128]
        vt = v.reshape(IMGS_PER_CORE, NCHUNK, 16, NWIN_IMG, SPLIT, C)
        vtok = np.ascontiguousarray(vt.transpose(2, 4, 0, 3, 1, 5)).reshape(
            128, NW * NCHUNK * C
        )

        # vpad: [c, win, 662] with interior at 12 + y*10 + x
        vimg = np.ascontiguousarray(v.transpose(4, 0, 2, 1, 3))  # [c, img, sx, y, x]
        vpad = np.zeros((C, NW, PADN), dtype=f16)
        vpad_v = vpad[:, :, 1:661].reshape(C, NW, RES + 2, XP)
        vpad_v[:, :, 1:-1, 1:-1] = vimg.reshape(C, NW, RES, SPLIT)

        cores.append(
            {
                "qT": qT.astype(f16),
                "kT": kT.astype(f16),
                "vtok": vtok.astype(f16),
                "vpad": vpad.reshape(C, NW * PADN),
            }
        )

    w9 = conv_w.reshape(C, 9).astype(np.float32)
    diag = np.zeros((C, 9, C), dtype=np.float32)
    idx = np.arange(C)
    for t in range(9):
        diag[idx, t, idx] = w9[:, t]
    diag = diag.reshape(C, 9 * C).astype(f16)
    for m in cores:
        m["diag"] = diag
        m["wtap"] = w9
    return cores


def kernel(qkv, conv_w, conv_b):
    from concourse.bass_utils import run_bass_kernel_spmd

    qkv = np.asarray(qkv, dtype=np.float32)
    conv_w = np.asarray(conv_w, np.float32)
    conv_b = np.asarray(conv_b, np.float32)

    nc = _get_nc()
    in_maps = _host_prep(qkv, conv_w)

    res = run_bass_kernel_spmd(nc, in_maps, core_ids=list(range(N_CORES)))
    global LAST_RESULT
    LAST_RESULT = res

    outs = []
    for r in res.results:
        av = r["avT"].astype(np.float32)       # [16, 128, 512]
        den = r["denT"].astype(np.float32)     # [16, 4, 512]
        lp = r["lepeT"].astype(np.float32)     # [16, 128, 512]
        att = av.reshape(NW, HEADS, HD, S) / den.reshape(NW, HEADS, 1, S)
        o = att.reshape(NW, C, S) + lp + conv_b.astype(np.float32)[None, :, None]
        # [win, c, s] -> [img, y, x, c]
        o = o.reshape(IMGS_PER_CORE, NWIN_IMG, C, RES, SPLIT)
        o = o.transpose(0, 3, 1, 4, 2).reshape(IMGS_PER_CORE, RES, RES, C)
        outs.append(o)
    return np.concatenate(outs, axis=0)


LAST_RESULT = None


# revision 37
# speedup vs baseline: 2.0150x; 1.4006x over previous
"""CSWin attention kernel for 8 trn2 NeuronCores — v3.

Strategy vs v2 (292us baseline):
  - host pre-casts qkv to f16 AND pre-transposes per window:
      qT/kT   [c=128, win, s=512]   (channel-major, contiguous big DMAs)
      vtok    [tok128, win, jc, c]  (token-major for AV stationary)
      vpad    [c, win, 662]         (host-padded LePE image, zero ring)
    -> no on-device casts, no PE transposes, 4-5 big contiguous input
       DMAs instead of 192 strided per-chunk DMAs (was ~118us of
       engine trigger time).
  - QK^T per chunk-half into PSUM f32 [128,1024] (2 heads row-tiled).
  - exp split across engines: ScalarE activation(Exp) for 5 of 8
    half-tiles per window, DVE Schraudolph bit-trick (i16 = s*A+B
    viewed as f16) for the other 3 -> breaks the 131us ScalarE floor.
  - AV + den(ones-matmul) accumulated over chunks in PSUM col-tiled.
  - LePE 3x3 depthwise conv: 9 diagonal-matmul taps with STRIDED
    moving view (exactly 512 interior cols, one phase, one PSUM bank);
    bias added on host.
  - outputs copied PSUM->SBUF f16 (DVE/ScalarE; GPSIMD can't touch
    PSUM) and DMA'd f16; host does att = av/den + lepe + bias and the
    window->image transpose.

PSUM budget: scores 2x[128,1024](4 banks) + av(1) + den(1) + lepe(2) = 8.
"""

import numpy as np

import concourse.bass as bass
import concourse.tile as tile
from concourse import mybir
from concourse.vector_clock import ScopedClock

RES = 64
SPLIT = 8
C = 128
HEADS = 4
HD = 32
S = RES * SPLIT          # 512 tokens per window
SCALE = HD ** -0.5
B = 16
N_CORES = 8
IMGS_PER_CORE = B // N_CORES   # 2
NWIN_IMG = RES // SPLIT        # 8 windows per image
NW = IMGS_PER_CORE * NWIN_IMG  # 16 windows per core
NCHUNK = S // 128              # 4 token-chunks per window

F32 = mybir.dt.float32
F16 = mybir.dt.float16
I16 = mybir.dt.int16

LOG2E = 1.4426950408889634
# Schraudolph f16: i16 = s * A + B, bit pattern of ~exp(s*SCALE)
SCH_A = float(SCALE * LOG2E * 1024.0)
SCH_B = float(15 * 1024 - 45 + 0.5)

# LePE padded image geometry (host-built): cell(y,x) = 12 + y*10 + x
XP = SPLIT + 2                 # 10
PADN = XP * (RES + 2) + 2      # 662
P0 = 12

# which half-tiles (idx = 2*jc + half, 0..7) use the DVE Schraudolph exp
DVE_HALVES = frozenset({1, 5})
TAPS = [(dy, dx) for dy in (-1, 0, 1) for dx in (-1, 0, 1)]
# LePE taps computed on the DVE (scalar_tensor_tensor chain) vs the PE
DVE_TAPS = (0, 1, 2)
PE_TAPS = tuple(t for t in range(9) if t not in DVE_TAPS)


# ---------------------------------------------------------------- compat ----

def _patched_drain_and_barrier(self, tick_clock, wait_clock):
    nc = self.nc
    nop_inst = nc.sync.nop(nofuse=True)
    wait_clock.add_sem_waits(nop_inst.ins, ScopedClock({None: tick_clock.global_clock}))
    si = nop_inst.ins.sync_info
    waits = list(si.on_wait) if si is not None else []
    if len(waits) > 1:
        si.on_wait = [waits[0]]
        for w in waits[1:]:
            n2 = nc.sync.nop(nofuse=True)
            n2.ins.sync_info = mybir.SyncInfo(on_wait=[w], on_update=[])
    nc.sync.drain()
    nc.all_engine_barrier()
    assert self.sems is not None
    popped = nc._tile_sem_poison_stack.pop()
    assert popped is self._sem_poison
    nc.clear_and_free_semaphores(list(self.sems.allocated().values()))
    nc.all_engine_barrier()


def _install_tile_patch():
    tile.TileContext._drain_and_barrier = _patched_drain_and_barrier


def _split_multiwaits(nc):
    """Hoist extra sync waits onto same-engine NOPs inserted just before the
    owning instruction (this walrus build allows 1 wait per instruction)."""
    for f in nc.m.functions:
        for bb in f.blocks:
            insts = bb.instructions
            if not any(
                i.sync_info is not None and len(i.sync_info.on_wait) > 1
                for i in insts
            ):
                continue
            new_insts = []
            for inst in insts:
                si = inst.sync_info
                if si is not None and len(si.on_wait) > 1:
                    waits = list(si.on_wait)
                    for w in waits[:-1]:
                        nop = mybir.InstNoOp(
                            name=nc.get_next_instruction_name(), ins=[], outs=[]
                        )
                        nop.engine = inst.engine
                        nop.sync_info = mybir.SyncInfo(on_wait=[w], on_update=[])
                        new_insts.append(nop)
                    si.on_wait = [waits[-1]]
                new_insts.append(inst)
            bb.instructions = new_insts


# ---------------------------------------------------------------- device ----

def _build_nc():
    _install_tile_patch()
    nc = bass.Bass(trn_type="TRN2", num_devices=N_CORES)

    AUGW = HD + 1            # 33: v columns + ones column (den rides along)
    AUGB = 2 * AUGW          # 66 aug columns per (chunk, bank)

    qT_d = nc.dram_tensor("qT", [C, NW * S], F16, kind="ExternalInput")
    kT_d = nc.dram_tensor("kT", [C, NW * S], F16, kind="ExternalInput")
    vtok_d = nc.dram_tensor(
        "vtok", [128, NW * NCHUNK * 2 * AUGB], F16, kind="ExternalInput"
    )
    vpad_d = nc.dram_tensor("vpad", [C, NW * PADN], F16, kind="ExternalInput")
    diag_d = nc.dram_tensor("diag", [C, 9 * C], F16, kind="ExternalInput")

    avA_d = nc.dram_tensor("avA", [NW, 98, S], F16, kind="ExternalOutput")
    avB_d = nc.dram_tensor("avB", [NW, 98, S], F16, kind="ExternalOutput")
    lp_d = nc.dram_tensor("lepeT", [NW, C, S], F16, kind="ExternalOutput")

    wtap_d = nc.dram_tensor("wtap", [C, 9], F32, kind="ExternalInput")

    GRP = 4  # windows per input-DMA group

    with tile.TileContext(nc) as tc:
        with (
            tc.tile_pool(name="const", bufs=1) as const,
            tc.tile_pool(name="inp", bufs=1) as inp,
            tc.tile_pool(name="expt", bufs=4) as expt,
            tc.tile_pool(name="sbout", bufs=2) as sbout,
            tc.tile_pool(name="lacc", bufs=2) as lacc,
            tc.tile_pool(name="scoresp", bufs=2, space="PSUM") as scoresp,
            tc.tile_pool(name="avp", bufs=1, space="PSUM") as avp,
            tc.tile_pool(name="lepep", bufs=2, space="PSUM") as lepep,
        ):
            diag_sb = const.tile([C, 9, C], F16)
            nc.sync.dma_start(
                out=diag_sb, in_=diag_d.ap().rearrange("c (t m) -> c t m", t=9)
            )
            ones_sb = const.tile([128, 1], F16)
            nc.vector.memset(ones_sb, 1.0)
            wtap_sb = const.tile([C, 9], F32)
            nc.sync.dma_start(out=wtap_sb, in_=wtap_d.ap())

            qT_sb = inp.tile([C, NW * S], F16)
            kT_sb = inp.tile([C, NW * S], F16)
            vtok_sb = inp.tile([128, NW * NCHUNK * 2 * AUGB], F16)
            vpad_sb = inp.tile([C, NW * PADN], F16)
            for g in range(NW // GRP):
                for t_sb, t_d, width in (
                    (qT_sb, qT_d, S),
                    (kT_sb, kT_d, S),
                    (vtok_sb, vtok_d, NCHUNK * 2 * AUGB),
                    (vpad_sb, vpad_d, PADN),
                ):
                    lo, hi = g * GRP * width, (g + 1) * GRP * width
                    nc.sync.dma_start(out=t_sb[:, lo:hi], in_=t_d.ap()[:, lo:hi])

            def vpad_view(w, t):
                dy, dx = TAPS[t]
                d = XP * dy + dx
                return bass.AP(
                    tensor=vpad_sb.tensor,
                    offset=vpad_sb.offset + w * PADN + P0 + d,
                    ap=[vpad_sb.ap[0], [XP, RES], [1, SPLIT]],
                )

            def lepe_taps(w):
                """PE diag-matmul taps into one PSUM bank + DVE STT taps into
                an SBUF f16 accumulator; returns (lp_psum, acc_sbuf)."""
                lp = lepep.tile([128, S], F32, tag="lp")
                for i, t in enumerate(PE_TAPS):
                    nc.tensor.matmul(
                        lp,
                        diag_sb[:, t, :],
                        vpad_view(w, t),
                        start=(i == 0),
                        stop=(i == len(PE_TAPS) - 1),
                        skip_group_check=True,
                    )
                acc = lacc.tile([128, S], F16, tag="lacc")
                for i, t in enumerate(DVE_TAPS):
                    if i == 0:
                        nc.vector.tensor_scalar(
                            out=acc,
                            in0=vpad_view(w, t),
                            scalar1=wtap_sb[:, t : t + 1],
                            scalar2=None,
                            op0=mybir.AluOpType.mult,
                        )
                    else:
                        nc.vector.scalar_tensor_tensor(
                            out=acc,
                            in0=vpad_view(w, t),
                            scalar=wtap_sb[:, t : t + 1],
                            in1=acc,
                            op0=mybir.AluOpType.mult,
                            op1=mybir.AluOpType.add,
                        )
                return lp, acc

            def lepe_out(w, lp, acc):
                # lepe = PE part (PSUM f32) + DVE part (SBUF f16)
                lp_sb = sbout.tile([128, S], F16, tag="lp_sb")
                nc.vector.scalar_tensor_tensor(
                    out=lp_sb,
                    in0=lp,
                    scalar=1.0,
                    in1=acc,
                    op0=mybir.AluOpType.mult,
                    op1=mybir.AluOpType.add,
                )
                nc.gpsimd.dma_start(out=lp_d.ap()[w], in_=lp_sb)

            prev_lp = None  # (w, lp tile, acc tile) of the previous window

            for w in range(NW):
                avA_ps = avp.tile([128, S], F32, tag="avA")
                avB_ps = avp.tile([128, S], F32, tag="avB")
                ets = {}

                def qk_exp(jc, half, w=w, ets=None):
                    st = scoresp.tile([128, 2 * S], F32, tag="st")
                    for hh in range(2):
                        h = 2 * half + hh
                        hp = 32 * h
                        nc.tensor.matmul(
                            st[:, S * hh : S * (hh + 1)],
                            kT_sb[hp : hp + 32, w * S + 128 * jc : w * S + 128 * (jc + 1)],
                            qT_sb[hp : hp + 32, w * S : (w + 1) * S],
                            start=True,
                            stop=True,
                            tile_position=(hp, 0),
                        )
                    et = expt.tile([128, 2 * S], F16, tag="et")
                    idx = 2 * jc + half
                    if idx in DVE_HALVES:
                        nc.vector.tensor_scalar(
                            out=et.bitcast(I16),
                            in0=st,
                            scalar1=SCH_A,
                            scalar2=SCH_B,
                            op0=mybir.AluOpType.mult,
                            op1=mybir.AluOpType.add,
                        )
                    else:
                        nc.scalar.activation(
                            out=et,
                            in_=st,
                            func=mybir.ActivationFunctionType.Exp,
                            scale=float(SCALE),
                        )
                    ets[idx] = et

                def av_den(jc, w=w, ets=None):
                    # per head one [K=128, M=33] matmul: v columns + a ones
                    # column, so row 32 of each 64-wide PE tile is the
                    # softmax denominator (no separate den matmuls)
                    base = (w * NCHUNK + jc) * 2 * AUGB
                    for h in range(HEADS):
                        bank = avA_ps if h < 2 else avB_ps
                        tc_ = 64 * (h % 2)
                        et_h = ets[2 * jc + h // 2][:, S * (h % 2) : S * (h % 2 + 1)]
                        nc.tensor.matmul(
                            bank[tc_ : tc_ + AUGW, :],
                            vtok_sb[:, base + AUGW * h : base + AUGW * (h + 1)],
                            et_h,
                            start=(jc == 0),
                            stop=(jc == NCHUNK - 1),
                            tile_position=(0, tc_),
                            skip_group_check=True,
                        )

                qk_exp(0, 0, ets=ets)
                qk_exp(0, 1, ets=ets)
                qk_exp(1, 0, ets=ets)
                qk_exp(1, 1, ets=ets)
                av_den(0, ets=ets)
                qk_exp(2, 0, ets=ets)
                qk_exp(2, 1, ets=ets)
                av_den(1, ets=ets)
                qk_exp(3, 0, ets=ets)
                qk_exp(3, 1, ets=ets)
                av_den(2, ets=ets)

                # drain previous window's lepe bank, then fill it for w;
                # the taps cover the PE while exp(3,*) drains
                if prev_lp is not None:
                    lepe_out(*prev_lp)
                prev_lp = (w, *lepe_taps(w))

                av_den(3, ets=ets)

                # ---- drain PSUM -> SBUF f16 -> DRAM ------------------------
                sbA = sbout.tile([128, S], F16, tag="sbA")
                nc.vector.tensor_copy(out=sbA, in_=avA_ps)
                nc.gpsimd.dma_start(out=avA_d.ap()[w], in_=sbA[0:98, :])

                sbB = sbout.tile([128, S], F16, tag="sbB")
                nc.vector.tensor_copy(out=sbB, in_=avB_ps)
                nc.gpsimd.dma_start(out=avB_d.ap()[w], in_=sbB[0:98, :])

            lepe_out(*prev_lp)

    _split_multiwaits(nc)
    return nc


# ------------------------------------------------------------------ host ----

_NC_CACHE = {}


def _get_nc():
    if "nc" not in _NC_CACHE:
        _NC_CACHE["nc"] = _build_nc()
    return _NC_CACHE["nc"]


def _host_prep(qkv, conv_w):
    """Build per-core input arrays (all f16)."""
    f16 = np.float16
    # [3, B, 4096, 128] -> window grids [3, B, y, sx, x, c]
    qkv_w = qkv.reshape(3, B, RES, NWIN_IMG, SPLIT, C)

    cores = []
    for core in range(N_CORES):
        bs = slice(core * IMGS_PER_CORE, (core + 1) * IMGS_PER_CORE)
        q = qkv_w[0, bs]   # [2, y, sx, x, c]
        k = qkv_w[1, bs]
        v = qkv_w[2, bs]

        # [c, img, sx, y, x] -> [128, NW*512]
        qT = np.ascontiguousarray(q.transpose(4, 0, 2, 1, 3)).reshape(C, NW * S)
        kT = np.ascontiguousarray(k.transpose(4, 0, 2, 1, 3)).reshape(C, NW * S)

        # v token-major, augmented with a ones column per head:
        # layout [tok128, win, jc, h, 33] with cols = [v_h (32) | 1]
        vt = v.reshape(IMGS_PER_CORE, NCHUNK, 16, NWIN_IMG, SPLIT, C)
        vtok = np.ascontiguousarray(vt.transpose(2, 4, 0, 3, 1, 5)).reshape(
            128, NW, NCHUNK, HEADS, HD
        )
        vaug = np.ones((128, NW, NCHUNK, HEADS, HD + 1), dtype=np.float32)
        vaug[..., :HD] = vtok
        vaug = vaug.reshape(128, NW * NCHUNK * HEADS * (HD + 1))

        # vpad: [c, win, 662] with interior at 12 + y*10 + x
        vimg = np.ascontiguousarray(v.transpose(4, 0, 2, 1, 3))  # [c, img, sx, y, x]
        vpad = np.zeros((C, NW, PADN), dtype=f16)
        vpad_v = vpad[:, :, 1:661].reshape(C, NW, RES + 2, XP)
        vpad_v[:, :, 1:-1, 1:-1] = vimg.reshape(C, NW, RES, SPLIT)

        cores.append(
            {
                "qT": qT.astype(f16),
                "kT": kT.astype(f16),
                "vtok": vaug.astype(f16),
                "vpad": vpad.reshape(C, NW * PADN),
            }
        )

    w9 = conv_w.reshape(C, 9).astype(np.float32)
    diag = np.zeros((C, 9, C), dtype=np.float32)
    idx = np.arange(C)
    for t in range(9):
        diag[idx, t, idx] = w9[:, t]
    diag = diag.reshape(C, 9 * C).astype(f16)
    for m in cores:
        m["diag"] = diag
        m["wtap"] = w9
    return cores


def kernel(qkv, conv_w, conv_b):
    from concourse.bass_utils import run_bass_kernel_spmd

    qkv = np.asarray(qkv, dtype=np.float32)
    conv_w = np.asarray(conv_w, np.float32)
    conv_b = np.asarray(conv_b, np.float32)

    nc = _get_nc()
    in_maps = _host_prep(qkv, conv_w)

    res = run_bass_kernel_spmd(nc, in_maps, core_ids=list(range(N_CORES)))
    global LAST_RESULT
    LAST_RESULT = res

    outs = []
    for r in res.results:
        avA = r["avA"].astype(np.float32)      # [16, 98, 512]
        avB = r["avB"].astype(np.float32)
        lp = r["lepeT"].astype(np.float32)     # [16, 128, 512]
        att = np.empty((NW, HEADS, HD, S), np.float32)
        for h, (bank, row) in enumerate(
            ((avA, 0), (avA, 64), (avB, 0), (avB, 64))
        ):
            att[:, h] = bank[:, row : row + HD] / bank[:, None, row + HD]
        o = att.reshape(NW, C, S) + lp + conv_b.astype(np.float32)[None, :, None]
        # [win, c, s] -> [img, y, x, c]
        o = o.reshape(IMGS_PER_CORE, NWIN_IMG, C, RES, SPLIT)
        o = o.transpose(0, 3, 1, 4, 2).reshape(IMGS_PER_CORE, RES, RES, C)
        outs.append(o)
    return np.concatenate(outs, axis=0)


LAST_RESULT = None
